# revision 18
# baseline (speedup 1.0000x reference)
"""Trainium2 Bass kernel for nn_Attention_4037269258732 (GQA attention with
RoPE, causal mask, and per-head sink-logit LSE renormalization).

Problem:  B=1, S=2048, DIM=2048, H=32 q-heads, KVH=8 kv-heads, HD=64.
          out = Wo @ attn(RoPE(Wq x), RoPE(Wk x), Wv x) + bo, causal,
          with out rows scaled by r = sumexp/(sumexp + e^sink).

Sharding (8 cores, tensor-parallel over heads):
  core c owns q-heads [4c, 4c+4), kv-head c, the matching rows of
  wq/wk/wv, wo's input-dim slice [256c, 256c+256), and sinks[4c:4c+4].
  Each core computes a full-shape [S, DIM] bf16 partial of the output
  projection; the host sums the 8 partials and adds wo_b once.

Device dataflow per core (feature dims on SBUF partitions so every
matmul chains without transposes; bf16 matmul operands / fp32 PSUM):
  qT[256,S], kT[64,S], vT[64,S] = W.T @ xT      (xT host-transposed)
  RoPE fused into PSUM eviction (rot_half via 32-partition-shifted DVE
  reads, sign folded into sin_rot); v_nat[S,64] via PE transpose.
  Attention per (block b of 512 sq, sk-tile t, head-pair hp):
    psq[128,1024] = [kT_lo.T @ q_lo | kT_hi.T @ q_hi]  (2 concurrent
    K=64 matmuls in disjoint PE row groups, 2 PSUM banks)
    ptt = exp(psq/8)   (single merged ACT instr for full tiles)
    PV col-packed: head 2hp -> pso[hp][0:64], head 2hp+1 -> [64:128]
    (concurrent col-group matmuls); per-head row sums via M=1
    ones-matmuls into one shared 'sums' bank at partitions {0,32,64,96}.
  Renorm per block: rowb = sums + e^sink (one DVE op), 1/r via
  exp(-ln(rowb)) on ACT, broadcast via K=128 bf16 select-matmuls,
  outstk = pso * rinv.  Output projection (2 K=128 matmuls per
  [128 sq, 512 d] tile) interleaved into the next block's t-loop;
  partials DMA'd out as bf16, bias added on the host.
"""

import numpy as np
import ml_dtypes

import bass_rust
import concourse.bass as bass
import concourse.tile as tile
from concourse import mybir
from concourse.bass_utils import run_bass_kernel_spmd

F32 = mybir.dt.float32
BF16 = mybir.dt.bfloat16
AF = mybir.ActivationFunctionType
OP = mybir.AluOpType
BF = ml_dtypes.bfloat16

B, S, DIM = 1, 2048, 2048
H, KVH, HD = 32, 8, 64
NCORES = 8
QH = H // NCORES          # 4 q heads per core
SBLK = 512                # sq block size
NSB = S // SBLK           # 4
NDC = DIM // 128          # 16 contraction chunks
NST = S // 128            # 16 sk tiles
SCALE = 1.0 / 8.0         # 1/sqrt(HD)

_ws_ctr = [0]


def _fix_range_clears(nc):
    """walrus here rejects the EVENT_SEMAPHORE_RANGE_CLEAR ISA struct
    ("ISA wrong length"); replace with per-sem write-0 NoOps."""
    import re as _re
    for f in nc.m.functions:
        for blk in f.blocks:
            out, changed = [], False
            for inst in blk.instructions:
                if type(inst).__name__ == "InstISA" and inst.isa_opcode == 176:
                    m = _re.search(r"range_first=(\d+) range_last=(\d+)", inst.concise())
                    first, last = int(m.group(1)), int(m.group(2))
                    for semid in range(first, last + 1):
                        _ws_ctr[0] += 1
                        nop = mybir.InstNoOp(name=f"I-rc-{_ws_ctr[0]}", ins=[], outs=[])
                        nop.engine = inst.engine
                        nop.sync_info = bass_rust.SyncInfo(
                            on_wait=[],
                            on_update=[
                                bass_rust.SyncUpdate(
                                    sync_type="semaphore",
                                    id=semid,
                                    update_mode="sem-wr-imm",
                                    update_value=0,
                                )
                            ],
                        )
                        out.append(nop)
                    changed = True
                    continue
                out.append(inst)
            if changed:
                blk.instructions = out


def _split_excess_waits(nc, max_waits=1):
    """walrus on this image encodes at most one SyncWait per instruction;
    hoist excess waits onto same-engine NoOps placed just before."""
    for f in nc.m.functions:
        for blk in f.blocks:
            out, changed = [], False
            for inst in blk.instructions:
                si = inst.sync_info
                waits = list(si.on_wait) if si is not None else []
                if len(waits) > max_waits:
                    excess, keep = waits[:-max_waits], waits[-max_waits:]
                    for k in range(0, len(excess), max_waits):
                        _ws_ctr[0] += 1
                        nop = mybir.InstNoOp(name=f"I-ws-{_ws_ctr[0]}", ins=[], outs=[])
                        nop.engine = inst.engine
                        nop.sync_info = bass_rust.SyncInfo(
                            on_wait=excess[k : k + max_waits], on_update=[]
                        )
                        out.append(nop)
                    inst.sync_info = bass_rust.SyncInfo(
                        on_wait=keep, on_update=list(si.on_update)
                    )
                    changed = True
                out.append(inst)
            if changed:
                blk.instructions = out


def prep_inputs(inputs):
    """Host-side sharding/layout prep. Returns per-core input maps."""
    x = np.asarray(inputs["x"], np.float32)
    rope = np.asarray(inputs["rope_cache"], np.float32)
    wq = np.asarray(inputs["wq_w"], np.float32)
    bq = np.asarray(inputs["wq_b"], np.float32)
    wk = np.asarray(inputs["wk_w"], np.float32)
    bk = np.asarray(inputs["wk_b"], np.float32)
    wv = np.asarray(inputs["wv_w"], np.float32)
    bv = np.asarray(inputs["wv_b"], np.float32)
    wo = np.asarray(inputs["wo_w"], np.float32)
    sinks = np.asarray(inputs["sinks"], np.float32)

    xT = np.ascontiguousarray(x[0].T).astype(BF)            # [DIM, S]
    cosT = rope[:, :HD].T                                   # [64, S]
    sinT = rope[:, HD:].T
    cos2 = np.ascontiguousarray(np.concatenate([cosT, cosT], 0)).astype(BF)
    # sin_rot indexed by DEST partition d: t2[d] = rot(q)[d]*sin[d] where
    # rot(q)[d] = -q[d+32] for d%64<32 else +q[d-32]; sign folded here.
    sr = np.concatenate([-sinT[0:32], sinT[32:64]], 0)      # [64, S]
    sin_rot2 = np.ascontiguousarray(np.concatenate([sr, sr], 0)).astype(BF)
    tri = np.triu(np.ones((128, 128), BF))                  # mask[p, j] = j >= p
    ident = np.eye(HD, dtype=BF)
    onesb = np.ones((128, 1), BF)
    # renorm broadcast selectors: rb01[p] = rowbinv[0] for p<64 else [32];
    # rb23[p] = rowbinv[64] for p<64 else [96]  (out = sel.T @ rowbinv)
    sel01 = np.zeros((128, 128), BF)
    sel01[0, 0:64] = 1
    sel01[32, 64:128] = 1
    sel23 = np.zeros((128, 128), BF)
    sel23[64, 0:64] = 1
    sel23[96, 64:128] = 1

    in_maps = []
    for c in range(NCORES):
        qs = slice(c * QH * HD, (c + 1) * QH * HD)          # 256 q rows
        ks = slice(c * HD, (c + 1) * HD)                    # 64 kv rows
        # wproj columns: [q 256 | k 64 | v 64] = 384
        wproj = np.concatenate([wq[qs].T, wk[ks].T, wv[ks].T], axis=1)
        bcol = np.zeros((128, 3), np.float32)
        bcol[:, 0] = bq[qs][0:128]
        bcol[:, 1] = bq[qs][128:256]
        bcol[0:64, 2] = bk[ks]
        bcol[64:128, 2] = bv[ks]
        # bias seen through the rotate-half permutation (32-block swap)
        rot = np.arange(128)
        rot = (rot // 64) * 64 + ((rot + 32) % 64)
        bcol_rot = bcol[rot]
        woT = np.ascontiguousarray(wo[:, qs].T).astype(BF)  # [256, DIM]
        esink_col = np.ones((128, 1), np.float32)
        for h in range(QH):
            esink_col[32 * h, 0] = np.exp(sinks[c * QH + h])
        in_maps.append(
            {
                "xT": xT,
                "wproj": np.ascontiguousarray(wproj).astype(BF),
                "bproj": bcol,
                "bprojr": bcol_rot,
                "cos2": cos2,
                "sinr2": sin_rot2,
                "woT": woT,
                "esinkc": esink_col,
                "tri": tri,
                "ident": ident,
                "onesb": onesb,
                "sel01": sel01,
                "sel23": sel23,
            }
        )
    return in_maps


def build_nc(split_waits=True):
    nc = bass.Bass("TRN2", target_bir_lowering=False, debug=False, num_devices=NCORES)
    xT = nc.dram_tensor("xT", [DIM, S], BF16, kind="ExternalInput").ap()
    wproj = nc.dram_tensor("wproj", [DIM, 384], BF16, kind="ExternalInput").ap()
    bproj = nc.dram_tensor("bproj", [128, 3], F32, kind="ExternalInput").ap()
    bprojr = nc.dram_tensor("bprojr", [128, 3], F32, kind="ExternalInput").ap()
    cos2 = nc.dram_tensor("cos2", [128, S], BF16, kind="ExternalInput").ap()
    sinr2 = nc.dram_tensor("sinr2", [128, S], BF16, kind="ExternalInput").ap()
    woT = nc.dram_tensor("woT", [2 * 128, DIM], BF16, kind="ExternalInput").ap()
    esinkc = nc.dram_tensor("esinkc", [128, 1], F32, kind="ExternalInput").ap()
    tri = nc.dram_tensor("tri", [128, 128], BF16, kind="ExternalInput").ap()
    ident = nc.dram_tensor("ident", [HD, HD], BF16, kind="ExternalInput").ap()
    onesb = nc.dram_tensor("onesb", [128, 1], BF16, kind="ExternalInput").ap()
    sel01 = nc.dram_tensor("sel01", [128, 128], BF16, kind="ExternalInput").ap()
    sel23 = nc.dram_tensor("sel23", [128, 128], BF16, kind="ExternalInput").ap()
    out = nc.dram_tensor("out", [S, DIM], BF16, kind="ExternalOutput").ap()

    with tile.TileContext(nc) as tc:
        with tc.tile_pool(name="persist", bufs=1) as P:
            # ---- long-lived SBUF tiles ----
            esink_t = P.tile([128, 1], F32, tag="esink")
            tri_t = P.tile([128, 128], BF16, tag="tri")
            wo_t = [
                P.tile([128, DIM], BF16, name=f"wo{i}", tag=f"wo{i}")
                for i in range(2)
            ]
            onesb_t = P.tile([128, 1], BF16, tag="onesb_t")
            sel01_t = P.tile([128, 128], BF16, tag="sel01")
            sel23_t = P.tile([128, 128], BF16, tag="sel23")
            # tiny dummy Exp/Ln to pull the ACT table load off the
            # attention critical path
            scr = P.tile([1, 16], F32, tag="scr")
            qp = [P.tile([128, S], BF16, name=f"qp{i}", tag=f"qp{i}") for i in range(2)]
            kT2 = P.tile([128, S], BF16, tag="kT2")
            vext = P.tile([128, NST * HD], BF16, tag="vext")
            outstk = [P.tile([128, S], BF16, name=f"os{i}", tag=f"os{i}") for i in range(2)]
            vT = P.tile([64, S], BF16, tag="vT")

            # ---- qkv projection, rope fused into eviction ----
            with (
                tc.tile_pool(name="projw", bufs=1) as PW,
                tc.tile_pool(name="tmp", bufs=2) as TMP,
                tc.tile_pool(name="psproj", bufs=2, space="PSUM") as PSP,
                tc.tile_pool(name="psv", bufs=2, space="PSUM") as PSV,
            ):
                x_t, w_t = [], []
                for dc in range(NDC):
                    wt = PW.tile([128, 384], BF16, name=f"w{dc}", tag=f"w{dc}")
                    nc.gpsimd.dma_start(wt[:], wproj[dc * 128 : (dc + 1) * 128, :])
                    w_t.append(wt)
                    xt = PW.tile([128, S], BF16, name=f"x{dc}", tag=f"x{dc}")
                    x_t.append(xt)
                # chunk-major x loads: evens on sync, chunk-0 odds on the
                # scalar queue (free until the first rope eviction), the
                # rest on sync behind the early chunks
                for dc in range(1, NDC, 2):
                    nc.scalar.dma_start(x_t[dc][:, 0:SBLK],
                                        xT[dc * 128 : (dc + 1) * 128, 0:SBLK])
                for ch in range(NSB):
                    cs = slice(ch * SBLK, (ch + 1) * SBLK)
                    for dc in range(0, NDC, 2):
                        nc.sync.dma_start(x_t[dc][:, cs],
                                          xT[dc * 128 : (dc + 1) * 128, cs])
                    if ch > 0:
                        for dc in range(1, NDC, 2):
                            nc.sync.dma_start(x_t[dc][:, cs],
                                              xT[dc * 128 : (dc + 1) * 128, cs])
                bcol_t = PW.tile([128, 3], F32, tag="bcol")
                nc.gpsimd.dma_start(bcol_t[:], bproj[:])
                bcolr_t = PW.tile([128, 3], F32, tag="bcolr")
                nc.gpsimd.dma_start(bcolr_t[:], bprojr[:])
                cos_t = PW.tile([128, S], BF16, tag="cos")
                nc.gpsimd.dma_start(cos_t[:], cos2[:])
                sinr_t = PW.tile([128, S], BF16, tag="sinr")
                nc.gpsimd.dma_start(sinr_t[:], sinr2[:])
                id_t = PW.tile([HD, HD], BF16, tag="ident")
                nc.gpsimd.dma_start(id_t[:], ident[:])
                nc.gpsimd.dma_start(onesb_t[:], onesb[:])
                nc.gpsimd.dma_start(esink_t[:], esinkc[:])
                nc.gpsimd.dma_start(tri_t[:], tri[:])
                nc.gpsimd.dma_start(sel01_t[:], sel01[:])
                nc.gpsimd.dma_start(sel23_t[:], sel23[:])
                for i in range(2):
                    nc.gpsimd.dma_start(
                        wo_t[i][:], woT[i * 128 : (i + 1) * 128, :]
                    )
                nc.scalar.activation(scr[0:1, 0:3], bcol_t[0:1, 0:3], AF.Exp)
                nc.scalar.activation(scr[0:1, 0:3], scr[0:1, 0:3], AF.Ln)

                for sb in range(NSB):
                    ss = slice(sb * SBLK, (sb + 1) * SBLK)
                    ps = [
                        PSP.tile([128, SBLK], F32, name=f"pp{j}", tag=f"pp{j}")
                        for j in range(3)
                    ]
                    for dc in range(NDC):
                        for j, (c0, c1) in enumerate(
                            [(0, 128), (128, 256), (256, 384)]
                        ):
                            nc.tensor.matmul(
                                ps[j][:],
                                w_t[dc][:, c0:c1],
                                x_t[dc][:, ss],
                                start=(dc == 0),
                                stop=(dc == NDC - 1),
                            )
                    # rope eviction: ScalarE copies PSUM->bf16 SBUF, DVE does
                    # the rotate-half as cheap bf16 copies (sign folded into
                    # sinr_t) plus two bf16 scalar_tensor_tensor + one add.
                    for i in range(2):
                        s1 = TMP.tile([128, SBLK], BF16, name="s1", tag="s1")
                        nc.scalar.add(s1[:], ps[i][:], bcol_t[:, i : i + 1])
                        s2 = TMP.tile([128, SBLK], BF16, name="s2", tag="s2")
                        for g in range(4):
                            d0 = 32 * g
                            s0 = 32 * g + 32 if g % 2 == 0 else 32 * g - 32
                            nc.vector.tensor_copy(
                                s2[d0 : d0 + 32, :], s1[s0 : s0 + 32, :]
                            )
                        t1 = TMP.tile([128, SBLK], BF16, name="t1", tag="t1")
                        nc.vector.tensor_tensor(
                            t1[:], s1[:], cos_t[:, ss], op=OP.mult
                        )
                        t2 = TMP.tile([128, SBLK], BF16, name="t2", tag="t2")
                        nc.vector.tensor_tensor(
                            t2[:], s2[:], sinr_t[:, ss], op=OP.mult
                        )
                        nc.vector.tensor_tensor(
                            qp[i][:, ss], t1[:], t2[:], op=OP.add
                        )
                    # k: rows 0:64 of ps[2]
                    s1k = TMP.tile([64, SBLK], BF16, name="s1k", tag="s1k")
                    nc.scalar.add(s1k[:], ps[2][0:64, :], bcol_t[0:64, 2:3])
                    s2k = TMP.tile([64, SBLK], BF16, name="s2k", tag="s2k")
                    nc.vector.tensor_copy(s2k[0:32, :], s1k[32:64, :])
                    nc.vector.tensor_copy(s2k[32:64, :], s1k[0:32, :])
                    tk1 = TMP.tile([64, SBLK], BF16, name="tk1", tag="tk1")
                    nc.vector.tensor_tensor(
                        tk1[:], s1k[:], cos_t[0:64, ss], op=OP.mult
                    )
                    tk2 = TMP.tile([64, SBLK], BF16, name="tk2", tag="tk2")
                    nc.vector.tensor_tensor(
                        tk2[:], s2k[:], sinr_t[0:64, ss], op=OP.mult
                    )
                    nc.vector.tensor_tensor(
                        kT2[0:64, ss], tk1[:], tk2[:], op=OP.add
                    )
                    nc.vector.tensor_copy(kT2[64:128, ss], kT2[0:64, ss])
                    # v: rows 64:128 of ps[2], bias only (ScalarE)
                    nc.scalar.add(
                        vT[:, ss], ps[2][64:128, :], bcol_t[64:128, 2:3]
                    )
                    # transpose this block's v tiles into vext
                    for t in range(4 * sb, 4 * sb + 4):
                        pv = PSV.tile([128, HD], BF16, name="pv", tag="pv")
                        nc.tensor.transpose(
                            pv[:], vT[:, t * 128 : (t + 1) * 128], id_t[:]
                        )
                        nc.vector.tensor_copy(
                            vext[:, t * HD : (t + 1) * HD], pv[:]
                        )

            # ---- attention + per-block renorm + interleaved output proj ----
            with (
                tc.tile_pool(name="psq", bufs=2, space="PSUM") as PSQ,
                tc.tile_pool(name="pso", bufs=1, space="PSUM") as PO,
                tc.tile_pool(name="psum_sums", bufs=1, space="PSUM") as PSUMS,
                tc.tile_pool(name="psf", bufs=1, space="PSUM") as PSF,
                tc.tile_pool(name="ptp", bufs=3) as PTP,
                tc.tile_pool(name="rows", bufs=2) as RP,
                tc.tile_pool(name="rbp", bufs=2) as RBP,
                tc.tile_pool(name="oev", bufs=4) as OE,
            ):
                sums_t = PSUMS.tile([128, SBLK], F32, tag="sums")
                nc.vector.memset(sums_t[:], 0.0)

                # outproj work queue: list of (st, db) for completed blocks
                oq = []

                _opi = [0]

                def emit_outproj(n, alt=False):
                    for _ in range(n):
                        if not oq:
                            return
                        st, db = oq.pop(0)
                        ds = slice(db * SBLK, (db + 1) * SBLK)
                        psf = PSF.tile([128, SBLK], F32, name="psf", tag="psf")
                        nc.tensor.matmul(
                            psf[:],
                            outstk[0][:, st * 128 : (st + 1) * 128],
                            wo_t[0][:, ds],
                            start=True,
                            stop=False,
                        )
                        nc.tensor.matmul(
                            psf[:],
                            outstk[1][:, st * 128 : (st + 1) * 128],
                            wo_t[1][:, ds],
                            start=False,
                            stop=True,
                        )
                        ot = OE.tile([128, SBLK], BF16, name="ot", tag="oe")
                        _opi[0] += 1
                        if alt and _opi[0] % 2 == 0:
                            nc.scalar.copy(ot[:], psf[:])
                        else:
                            nc.vector.tensor_copy(ot[:], psf[:])
                        nc.sync.dma_start(out[st * 128 : (st + 1) * 128, ds], ot[:])

                for b in range(NSB):
                    bs = b * SBLK
                    nt = 4 * b + 4
                    pso = [
                        PO.tile([128, SBLK], F32, name=f"po{i}", tag=f"po{i}")
                        for i in range(2)
                    ]

                    def emit_pv(t, hp, ptt, off):
                        # PV col-packed: head 2hp -> partitions 0:64,
                        # head 2hp+1 -> partitions 64:128 of pso[hp]
                        nc.tensor.matmul(
                            pso[hp][0:64, off:SBLK],
                            vext[:, t * HD : (t + 1) * HD],
                            ptt[:, off:SBLK],
                            start=(t == 0),
                            stop=(t == nt - 1),
                            tile_position=(0, 0),
                        )
                        nc.tensor.matmul(
                            pso[hp][64:128, off:SBLK],
                            vext[:, t * HD : (t + 1) * HD],
                            ptt[:, SBLK + off : 2 * SBLK],
                            start=(t == 0),
                            stop=(t == nt - 1),
                            skip_group_check=True,
                            tile_position=(0, 64),
                        )
                        # per-head row sums into the shared sums bank at
                        # partitions {0,32,64,96}; each head's first write
                        # of the block clears its own partition (start).
                        for lane in range(2):
                            h = 2 * hp + lane
                            nc.tensor.matmul(
                                sums_t[32 * h : 32 * h + 1, off:SBLK],
                                onesb_t[:, 0:1],
                                ptt[:, lane * SBLK + off : lane * SBLK + SBLK],
                                start=(t == 0),
                                stop=(t == nt - 1),
                                skip_group_check=True,
                                tile_position=(0, 32 * h),
                            )

                    niter = 2 * nt
                    prev = None
                    for it in range(niter):
                        t, hp = it // 2, it % 2
                        off = 128 * (t - 4 * b) if t >= 4 * b else 0
                        tc0, tc1 = t * 128, (t + 1) * 128
                        psq = PSQ.tile([128, 2 * SBLK], F32, name="psq", tag="psq")
                        nc.tensor.matmul(
                            psq[:, off:SBLK],
                            kT2[0:64, tc0:tc1],
                            qp[hp][0:64, bs + off : bs + SBLK],
                            start=True, stop=True,
                            tile_position=(0, 0),
                        )
                        nc.tensor.matmul(
                            psq[:, SBLK + off : 2 * SBLK],
                            kT2[64:128, tc0:tc1],
                            qp[hp][64:128, bs + off : bs + SBLK],
                            start=True, stop=True,
                            tile_position=(64, 0),
                        )
                        ptt = PTP.tile([128, 2 * SBLK], BF16, name="ptt", tag="pt")
                        if off == 0:
                            nc.scalar.activation(
                                ptt[:], psq[:], AF.Exp, scale=SCALE,
                            )
                        else:
                            nc.scalar.activation(
                                ptt[:, off:SBLK], psq[:, off:SBLK],
                                AF.Exp, scale=SCALE,
                            )
                            nc.scalar.activation(
                                ptt[:, SBLK + off : 2 * SBLK],
                                psq[:, SBLK + off : 2 * SBLK],
                                AF.Exp, scale=SCALE,
                            )
                        if t >= 4 * b:
                            nc.vector.tensor_tensor(
                                ptt[:, off : off + 128],
                                ptt[:, off : off + 128],
                                tri_t[:], op=OP.mult,
                            )
                            nc.vector.tensor_tensor(
                                ptt[:, SBLK + off : SBLK + off + 128],
                                ptt[:, SBLK + off : SBLK + off + 128],
                                tri_t[:], op=OP.mult,
                            )
                        # outproj filler while ACT chews on exp(it)
                        if it >= 2:
                            emit_outproj(2 if len(oq) > niter - it else 1)
                        # lagged PV of the previous iteration (its exp done)
                        if prev is not None:
                            emit_pv(*prev)
                        prev = (t, hp, ptt, off)
                    emit_pv(*prev)
                    # ---- renorm ----
                    rowb = RP.tile([128, SBLK], F32, name="rowb", tag="rowb")
                    nc.vector.tensor_scalar_add(rowb[:], sums_t[:], esink_t[:, 0:1])
                    rl = RP.tile([128, SBLK], F32, name="rl", tag="rl")
                    nc.scalar.activation(rl[:], rowb[:], AF.Ln)
                    rinv = RP.tile([128, SBLK], BF16, name="rinv", tag="rinv")
                    nc.scalar.activation(rinv[:], rl[:], AF.Exp, scale=-1.0)
                    for i, sel in ((0, sel01_t), (1, sel23_t)):
                        ps_rb = PSF.tile([128, SBLK], F32, name="psrb", tag="psf")
                        nc.tensor.matmul(
                            ps_rb[:], sel[:], rinv[:], start=True, stop=True,
                        )
                        rb = RBP.tile([128, SBLK], BF16, name=f"rb{i}", tag=f"rb{i}")
                        nc.vector.tensor_copy(rb[:], ps_rb[:])
                        nc.vector.tensor_tensor(
                            outstk[i][:, bs : bs + SBLK], pso[i][:], rb[:],
                            op=OP.mult,
                        )
                    for st in range(4 * b, 4 * b + 4):
                        for db in range(NSB):
                            oq.append((st, db))
                # drain remaining outproj work
                emit_outproj(len(oq), alt=True)

    _fix_range_clears(nc)
    if split_waits:
        _split_excess_waits(nc)
    return nc


_nc_cache = [None]


def kernel(**inputs):
    in_maps = prep_inputs(inputs)
    if _nc_cache[0] is None:
        _nc_cache[0] = build_nc()
    nc = _nc_cache[0]
    res = run_bass_kernel_spmd(nc, in_maps, list(range(NCORES)))
    acc = res.results[0]["out"].astype(np.float32)
    for i in range(1, NCORES):
        acc = acc + res.results[i]["out"].astype(np.float32)
    acc = acc + np.asarray(inputs["wo_b"], np.float32)[None, :]
    return acc.reshape(B, S, DIM)
